# revision 3
# baseline (speedup 1.0000x reference)
"""Causal self-attention (B=4, T=2048, C=1024, H=16) on 8 TRN2 NeuronCores.

Sharding: core = (batch b, head-group hg). Data parallel over B (4), tensor
parallel over heads (2 groups of 8). Each core computes a partial output
projection for its 8 heads; the host sums the two partials per batch
(row-parallel linear unshard).

v2 design (vs the fp32r baseline):
  - x is transposed and cast to bf16 on the HOST (xT [C, T]); all matmul
    operands are bf16/f16 (fp32 psum accumulate).  This removes the 128 PE
    transposes, halves input DMA, and enables FWL fast weight loads.
  - Software-pipelined emission: k/v/q generation, attention per 512-wide q
    superblock (qsb), and the output projection are interleaved so the
    scalar-engine exp stream overlaps the qkv/proj matmuls.  Leftover
    generation work is emitted as "fillers" between attention heads so the
    PE never starves while exp catches up.
  - Scalar engine runs ONLY exp (plus a few early staging copies); all other
    copies go to vector, masks + partition broadcasts to gpsimd.
  - Diagonal score blocks are packed tightly in the psum wide tile so one
    exp ACTIVATE covers a whole pair with no padding columns.
  - Softmax division: vector.reciprocal straight off the staged bf16
    denominator row, gpsimd partition_broadcast, one 2-byte-mode vector mul.

Per-core pipeline (per head h of 8, head dim 64, all psum fp32):
  kT [512ch, T] and qT (zero-padded per head to K=128) from wk/wq^T xT,
  v_aug [T, 8h, 65] (ones column -> softmax denominator from the PE),
  scoresT [k, q] = kT^T qT per 128-k-block, p = exp(scoresT/32) (|s|<=~2.1,
  no max subtraction), causal: skip blocks above the diagonal, width-trim +
  triangular-mask the 4 diagonal blocks, yT_aug [65, 512q] += v_aug^T p,
  yT = yT_aug[0:64] * (1/denominator), out [T, 1024] = yT^T wp.
"""
import numpy as np
from contextlib import ExitStack

import concourse.bass as bass
import concourse.mybir as mybir
import concourse.tile as tile
from concourse import bacc
from concourse.bass_utils import run_bass_kernel_spmd

import ml_dtypes

F32 = mybir.dt.float32
BF16 = mybir.dt.bfloat16
F16 = mybir.dt.float16
AF = mybir.ActivationFunctionType

T = 2048
C = 1024
H_PER_CORE = 8          # heads per core
D = 64                  # head dim
GC = H_PER_CORE * D     # 512 channels per head-group
SCALE = 1.0 / 32.0      # C ** -0.5
N_CORES = 8
NCT = C // 128          # 8 c-tiles
NT = T // 128           # 16 t-tiles
NQSB = T // 512         # 4 q superblocks


def build(nc):
    xT_d = nc.dram_tensor("xT", [C, T], BF16, kind="ExternalInput").ap()
    wq_d = nc.dram_tensor("wq", [C, GC], BF16, kind="ExternalInput").ap()
    wk_d = nc.dram_tensor("wk", [C, GC], BF16, kind="ExternalInput").ap()
    wv_d = nc.dram_tensor("wv", [C, GC], BF16, kind="ExternalInput").ap()
    wp_d = nc.dram_tensor("wp", [GC, C], BF16, kind="ExternalInput").ap()
    out_d = nc.dram_tensor("out", [T, C], F32, kind="ExternalOutput").ap()

    with tile.TileContext(nc) as tc, ExitStack() as ctx:
        const = ctx.enter_context(tc.tile_pool(name="const", bufs=1))
        persist = ctx.enter_context(tc.tile_pool(name="persist", bufs=1))

        # tri_mask[k, j] = 1.0 if k <= j else 0.0
        tri_mask = const.tile([128, 128], F16)
        nc.gpsimd.memset(tri_mask[:], 1.0)
        nc.gpsimd.affine_select(
            out=tri_mask[:], in_=tri_mask[:],
            compare_op=mybir.AluOpType.is_ge, fill=0.0, base=0,
            pattern=[[1, 128]], channel_multiplier=-1,
        )

        # persistent activations / weights
        xT = persist.tile([128, NCT, T], BF16)             # [c-tile, t]
        kT_sb = persist.tile([128, 4, T], BF16)            # [m-tile, t]
        v_aug = persist.tile([128, H_PER_CORE, NT, 65], F16)
        # q ring: slot = qsb % 2; head h occupies partition half (h%2)*64,
        # the other half stays zero (K=128 scores matmul, pad rows kill the
        # other head's kT contribution).
        qT_ring = persist.tile([128, 2, H_PER_CORE, 512], BF16)
        yT_sb = persist.tile([128, 4, T], BF16)
        wq_sb = persist.tile([128, NCT, GC], BF16)
        wk_sb = persist.tile([128, NCT, GC], BF16)
        wv_sb = persist.tile([128, NCT, GC], BF16)
        wp_sb = persist.tile([128, 4, C], BF16)

        nc.gpsimd.memset(qT_ring[:], 0.0)
        nc.gpsimd.memset(v_aug[:, :, :, 64], 1.0)

        # input DMAs; wk/xT interleaved so k-gen can start ASAP
        for ct in range(NCT):
            nc.sync.dma_start(wk_sb[:, ct, :], wk_d[ct * 128:(ct + 1) * 128, :])
            nc.sync.dma_start(xT[:, ct, :], xT_d[ct * 128:(ct + 1) * 128, :])
        for ct in range(NCT):
            nc.sync.dma_start(wv_sb[:, ct, :], wv_d[ct * 128:(ct + 1) * 128, :])
        for ct in range(NCT):
            nc.sync.dma_start(wq_sb[:, ct, :], wq_d[ct * 128:(ct + 1) * 128, :])
        for kt in range(4):
            nc.sync.dma_start(wp_sb[:, kt, :], wp_d[kt * 128:(kt + 1) * 128, :])

        pT_pool = ctx.enter_context(tc.tile_pool(name="pT", bufs=6))
        ySt_pool = ctx.enter_context(tc.tile_pool(name="ySt", bufs=4))
        rc_pool = ctx.enter_context(tc.tile_pool(name="rc", bufs=3))
        rb_pool = ctx.enter_context(tc.tile_pool(name="rb", bufs=3))
        so_pool = ctx.enter_context(tc.tile_pool(name="so", bufs=3))
        ps = ctx.enter_context(tc.tile_pool(name="ps", bufs=1, space="PSUM"))
        # psum budget: wide 3*[128,1024] (6 banks) + yTp 2*[65,512] (2 banks)

        def gen_k(half):
            """kT for t in [half*1024, (half+1)*1024), all 4 m-tiles."""
            for mt in range(4):
                wide = ps.tile([128, 1024], F32, tag="wide", bufs=3)
                for c2 in range(2):
                    chunk = half * 2 + c2
                    for ct in range(NCT):
                        nc.tensor.matmul(
                            wide[:, c2 * 512:(c2 + 1) * 512],
                            wk_sb[:, ct, mt * 128:(mt + 1) * 128],
                            xT[:, ct, chunk * 512:(chunk + 1) * 512],
                            start=(ct == 0), stop=(ct == NCT - 1))
                nc.vector.tensor_copy(
                    kT_sb[:, mt, half * 1024:(half + 1) * 1024], wide[:])

        def gen_v(tp):
            """v for t-tiles 2*tp, 2*tp+1 into v_aug."""
            wide = ps.tile([128, 1024], F32, tag="wide", bufs=3)
            for i in range(2):
                tt = 2 * tp + i
                for ct in range(NCT):
                    nc.tensor.matmul(
                        wide[:, i * 512:(i + 1) * 512],
                        xT[:, ct, tt * 128:(tt + 1) * 128],
                        wv_sb[:, ct, :],
                        start=(ct == 0), stop=(ct == NCT - 1))
            nc.vector.tensor_copy(
                v_aug[:, :, 2 * tp:2 * tp + 2, 0:64],
                wide[:].rearrange("p (i h d) -> p h i d", i=2, h=H_PER_CORE))

        def gen_q(qsb):
            """qT (padded layout) for q in [qsb*512, (qsb+1)*512)."""
            ring = qsb % 2
            for w in range(2):
                wide = ps.tile([128, 1024], F32, tag="wide", bufs=3)
                for i in range(2):
                    mt = 2 * w + i
                    for ct in range(NCT):
                        nc.tensor.matmul(
                            wide[:, i * 512:(i + 1) * 512],
                            wq_sb[:, ct, mt * 128:(mt + 1) * 128],
                            xT[:, ct, qsb * 512:(qsb + 1) * 512],
                            start=(ct == 0), stop=(ct == NCT - 1))
                # heads 4w,4w+2 live in partitions 0:64; 4w+1,4w+3 in 64:128
                src = wide[:].rearrange("p (i t) -> p i t", i=2)
                nc.vector.tensor_copy(
                    qT_ring[0:64, ring, 4 * w:4 * w + 3:2, :], src[0:64])
                nc.vector.tensor_copy(
                    qT_ring[64:128, ring, 4 * w + 1:4 * w + 4:2, :], src[64:128])

        def attention(qsb, fillers=()):
            fillers = list(fillers)
            ring = qsb % 2
            nkb = 4 * (qsb + 1)
            for h in range(H_PER_CORE):
                mt_h = h // 2
                yTp = ps.tile([65, 512], F32, tag="yTp", bufs=2)
                prefix = [(kb, 0) for kb in range(4 * qsb)]
                diag = [(kb, kb * 128 - qsb * 512)
                        for kb in range(4 * qsb, nkb)]
                groups = [prefix[i:i + 2] for i in range(0, len(prefix), 2)]
                groups += [diag[0:2], diag[2:4]]
                n_pv = 0
                for g in groups:
                    diag_group = g[0][0] >= 4 * qsb
                    wide = ps.tile([128, 1024], F32, tag="wide", bufs=3)
                    pTw = pT_pool.tile([128, 1024], F16, tag="pTw")
                    # pack blocks tightly: diag blocks are width-trimmed to
                    # their causal range and packed back-to-back so one exp
                    # ACTIVATE covers the pair without padding columns.
                    offs, widths = [], []
                    off = 0
                    for kb, lo in g:
                        offs.append(off)
                        widths.append(512 - lo)
                        off += 512 - lo
                    for i, (kb, lo) in enumerate(g):
                        nc.tensor.matmul(
                            wide[:, offs[i]:offs[i] + widths[i]],
                            kT_sb[:, mt_h, kb * 128:(kb + 1) * 128],
                            qT_ring[:, ring, h, lo:512],
                            start=True, stop=True)
                    nc.scalar.activation(
                        pTw[:, 0:off], wide[:, 0:off],
                        AF.Exp, bias=0.0, scale=SCALE)
                    for i, (kb, lo) in enumerate(g):
                        if diag_group:
                            # zero the strictly-upper triangle (first 128
                            # cols of the trimmed block)
                            nc.gpsimd.tensor_mul(
                                pTw[:, offs[i]:offs[i] + 128],
                                pTw[:, offs[i]:offs[i] + 128],
                                tri_mask[:])
                        nc.tensor.matmul(
                            yTp[:, lo:512],
                            v_aug[:, h, kb, :],
                            pTw[:, offs[i]:offs[i] + widths[i]],
                            start=(n_pv == 0), stop=(n_pv == nkb - 1))
                        n_pv += 1
                # softmax division: stage yT_aug out of psum (frees the
                # bank), then recip/broadcast/mul entirely in SBUF.
                ySt = ySt_pool.tile([65, 512], BF16, tag="ySt")
                nc.vector.tensor_copy(ySt[:], yTp[:])
                recip = rc_pool.tile([1, 512], BF16, tag="recip")
                with nc.allow_low_precision("bf16 softmax reciprocal"):
                    nc.vector.reciprocal(recip[:], ySt[64:65, :])
                rbc = rb_pool.tile([64, 512], BF16)
                nc.gpsimd.partition_broadcast(rbc[:], recip[:])
                nc.vector.tensor_mul(
                    yT_sb[64 * (h % 2):64 * (h % 2) + 64, mt_h,
                          qsb * 512:(qsb + 1) * 512],
                    ySt[0:64, :], rbc[:])
                if fillers:
                    fillers.pop(0)()
            for f in fillers:
                f()

        def proj(qsb):
            """out rows [qsb*512, (qsb+1)*512) = yT^T wp."""
            for j in range(4):
                tt = qsb * 4 + j
                wide = ps.tile([128, 1024], F32, tag="wide", bufs=3)
                for n2 in range(2):
                    for kt in range(4):
                        nc.tensor.matmul(
                            wide[:, n2 * 512:(n2 + 1) * 512],
                            yT_sb[:, kt, tt * 128:(tt + 1) * 128],
                            wp_sb[:, kt, n2 * 512:(n2 + 1) * 512],
                            start=(kt == 0), stop=(kt == 3))
                so = so_pool.tile([128, C], F32)
                if qsb == 3:
                    nc.scalar.copy(so[:], wide[:])
                else:
                    nc.vector.tensor_copy(so[:], wide[:])
                nc.sync.dma_start(
                    out_d[tt * 128:(tt + 1) * 128, :], so[:])

        # ---- emission schedule (software pipeline) ----
        gen_k(0)
        gen_v(0)
        gen_v(1)
        gen_q(0)
        attention(0, fillers=[
            lambda: gen_q(1),
            lambda: gen_k(1),
            lambda: gen_v(2),
            lambda: gen_v(3),
        ])
        proj(0)
        attention(1, fillers=[
            lambda: gen_q(2),
            lambda: gen_v(4),
            lambda: gen_v(5),
            lambda: gen_v(6),
            lambda: gen_v(7),
        ])
        proj(1)
        attention(2, fillers=[lambda: gen_q(3)])
        proj(2)
        attention(3)
        proj(3)


_CACHE = {}


def _get_nc():
    if "nc" not in _CACHE:
        nc = bacc.Bacc("TRN2", target_bir_lowering=False, debug=False,
                       num_devices=N_CORES)
        build(nc)
        nc.compile()
        _CACHE["nc"] = nc
    return _CACHE["nc"]


def make_in_maps(x, w_attn, w_proj):
    x = np.asarray(x, dtype=np.float32)
    w_attn = np.asarray(w_attn, dtype=np.float32)
    w_proj = np.asarray(w_proj, dtype=np.float32)
    BF = ml_dtypes.bfloat16
    in_maps = []
    for core in range(N_CORES):
        b, hg = divmod(core, 2)
        cs = slice(hg * GC, (hg + 1) * GC)
        in_maps.append({
            "xT": np.ascontiguousarray(x[b].T.astype(BF)),
            "wq": np.ascontiguousarray(w_attn[:, 0 * C:1 * C][:, cs].astype(BF)),
            "wk": np.ascontiguousarray(w_attn[:, 1 * C:2 * C][:, cs].astype(BF)),
            "wv": np.ascontiguousarray(w_attn[:, 2 * C:3 * C][:, cs].astype(BF)),
            "wp": np.ascontiguousarray(w_proj[cs, :].astype(BF)),
        })
    return in_maps


def kernel(x, w_attn, w_proj, _trace=False, _trace_kwargs=None):
    nc = _get_nc()
    in_maps = make_in_maps(x, w_attn, w_proj)
    res = None
    for attempt in range(3):
        try:
            res = run_bass_kernel_spmd(nc, in_maps,
                                       core_ids=list(range(N_CORES)),
                                       trace=_trace, **(_trace_kwargs or {}))
            break
        except Exception:
            # a previous process can leave the device wedged
            # (NRT_EXEC_UNIT_UNRECOVERABLE); a retry recovers it
            if attempt == 2:
                raise
    _CACHE["last_results"] = res
    B = np.asarray(x).shape[0]
    out = np.empty((B, T, C), dtype=np.float32)
    for b in range(B):
        out[b] = res.results[2 * b]["out"] + res.results[2 * b + 1]["out"]
    return out


# revision 6
# speedup vs baseline: 2.1686x; 2.1686x over previous
"""Causal self-attention (B=4, T=2048, C=1024, H=16) on 8 TRN2 NeuronCores.

Sharding: core = (batch b, head-group hg). Data parallel over B (4), tensor
parallel over heads (2 groups of 8). Each core computes a partial output
projection for its 8 heads; the host sums the two partials per batch
(row-parallel linear unshard).

v3 design (vs the fp32r baseline):
  - x is transposed and cast to bf16 on the HOST (xT [C, T]); all matmul
    operands are bf16/f16 (fp32 psum accumulate).  This removes the 128 PE
    transposes, halves input DMA, and enables FWL fast weight loads.
  - Software-pipelined emission: k/v/q generation, attention per 512-wide q
    superblock (qsb), and the output projection are interleaved so the
    scalar-engine exp stream overlaps the qkv/proj matmuls.  Leftover
    generation work is emitted as "fillers" between attention heads so the
    PE never starves while exp catches up.
  - Cross-group pipelining inside attention: the p@v matmuls of group g are
    emitted after the scores matmuls of group g+1, so the exp->mask latency
    hides behind real PE work instead of stalling the in-order PE queue.
  - Diagonal score blocks are packed tightly in the psum wide tile so one
    exp ACTIVATE covers a whole pair with no padding columns.
  - Engines: scalar = exp only; vector = staging casts, masks, softmax
    division; gpsimd = partition broadcasts + memsets only (its FIFO is
    shallow and slow to resume, so it must never sit between PE-critical
    producers and consumers).

Per-core pipeline (per head h of 8, head dim 64, all psum fp32):
  kT [512ch, T] and qT (zero-padded per head to K=128) from wk/wq^T xT,
  v_aug [T, 8h, 65] (ones column -> softmax denominator from the PE),
  scoresT [k, q] = kT^T qT per 128-k-block, p = exp(scoresT/32) (|s|<=~2.1,
  no max subtraction), causal: skip blocks above the diagonal, width-trim +
  triangular-mask the 4 diagonal blocks, yT_aug [65, 512q] += v_aug^T p,
  yT = yT_aug[0:64] * (1/denominator), out [T, 1024] = yT^T wp.
"""
import numpy as np
from contextlib import ExitStack

import concourse.bass as bass
import concourse.mybir as mybir
import concourse.tile as tile
from concourse import bacc
from concourse.bass_utils import run_bass_kernel_spmd

import ml_dtypes

F32 = mybir.dt.float32
BF16 = mybir.dt.bfloat16
F16 = mybir.dt.float16
AF = mybir.ActivationFunctionType

T = 2048
C = 1024
H_PER_CORE = 8          # heads per core
D = 64                  # head dim
GC = H_PER_CORE * D     # 512 channels per head-group
SCALE = 1.0 / 32.0      # C ** -0.5
N_CORES = 8
NCT = C // 128          # 8 c-tiles
NT = T // 128           # 16 t-tiles
NQSB = T // 512         # 4 q superblocks


def build(nc):
    xT_d = nc.dram_tensor("xT", [C, T], BF16, kind="ExternalInput").ap()
    wq_d = nc.dram_tensor("wq", [C, GC], BF16, kind="ExternalInput").ap()
    wk_d = nc.dram_tensor("wk", [C, GC], BF16, kind="ExternalInput").ap()
    wv_d = nc.dram_tensor("wv", [C, GC], BF16, kind="ExternalInput").ap()
    wp_d = nc.dram_tensor("wp", [GC, C], BF16, kind="ExternalInput").ap()
    out_d = nc.dram_tensor("out", [T, C], F32, kind="ExternalOutput").ap()

    with tile.TileContext(nc) as tc, ExitStack() as ctx:
        const = ctx.enter_context(tc.tile_pool(name="const", bufs=1))
        persist = ctx.enter_context(tc.tile_pool(name="persist", bufs=1))

        # tri_mask[k, j] = 1.0 if k <= j else 0.0
        tri_mask = const.tile([128, 128], F16)
        nc.gpsimd.memset(tri_mask[:], 1.0)
        nc.gpsimd.affine_select(
            out=tri_mask[:], in_=tri_mask[:],
            compare_op=mybir.AluOpType.is_ge, fill=0.0, base=0,
            pattern=[[1, 128]], channel_multiplier=-1,
        )

        # persistent activations / weights
        xT = persist.tile([128, NCT, T], BF16)             # [c-tile, t]
        kT_sb = persist.tile([128, 4, T], BF16)            # [m-tile, t]
        v_aug = persist.tile([128, H_PER_CORE, NT, 65], F16)
        # q ring: slot = qsb % 2; head h occupies partition half (h%2)*64,
        # the other half stays zero (K=128 scores matmul, pad rows kill the
        # other head's kT contribution).
        qT_ring = persist.tile([128, 2, H_PER_CORE, 512], BF16)
        yT_sb = persist.tile([128, 4, T], BF16)
        wq_sb = persist.tile([128, NCT, GC], BF16)
        wk_sb = persist.tile([128, NCT, GC], BF16)
        wv_sb = persist.tile([128, NCT, GC], BF16)
        wp_sb = persist.tile([128, 4, C], BF16)

        nc.gpsimd.memset(qT_ring[:], 0.0)
        nc.gpsimd.memset(v_aug[:, :, :, 64], 1.0)

        # input DMAs; wk/xT interleaved so k-gen can start ASAP
        for ct in range(NCT):
            nc.sync.dma_start(wk_sb[:, ct, :], wk_d[ct * 128:(ct + 1) * 128, :])
            nc.sync.dma_start(xT[:, ct, :], xT_d[ct * 128:(ct + 1) * 128, :])
        for ct in range(NCT):
            nc.sync.dma_start(wv_sb[:, ct, :], wv_d[ct * 128:(ct + 1) * 128, :])
        for ct in range(NCT):
            nc.sync.dma_start(wq_sb[:, ct, :], wq_d[ct * 128:(ct + 1) * 128, :])
        for kt in range(4):
            nc.sync.dma_start(wp_sb[:, kt, :], wp_d[kt * 128:(kt + 1) * 128, :])

        pT_pool = ctx.enter_context(tc.tile_pool(name="pT", bufs=6))
        rc_pool = ctx.enter_context(tc.tile_pool(name="rc", bufs=3))
        rb_pool = ctx.enter_context(tc.tile_pool(name="rb", bufs=3))
        so_pool = ctx.enter_context(tc.tile_pool(name="so", bufs=3))
        ps = ctx.enter_context(tc.tile_pool(name="ps", bufs=1, space="PSUM"))
        # psum budget: wide 3*[128,1024] (6 banks) + yTp 2*[65,512] (2 banks)

        so_cnt = [0]

        def gen_k(half, mts):
            """kT for t in [half*1024, (half+1)*1024) for the given m-tiles."""
            for mt in mts:
                wide = ps.tile([128, 1024], F32, tag="wide", bufs=3)
                for c2 in range(2):
                    chunk = half * 2 + c2
                    for ct in range(NCT):
                        nc.tensor.matmul(
                            wide[:, c2 * 512:(c2 + 1) * 512],
                            wk_sb[:, ct, mt * 128:(mt + 1) * 128],
                            xT[:, ct, chunk * 512:(chunk + 1) * 512],
                            start=(ct == 0), stop=(ct == NCT - 1))
                nc.vector.tensor_copy(
                    kT_sb[:, mt, half * 1024:(half + 1) * 1024], wide[:])

        def gen_v(tp):
            """v for t-tiles 2*tp, 2*tp+1 into v_aug."""
            wide = ps.tile([128, 1024], F32, tag="wide", bufs=3)
            for i in range(2):
                tt = 2 * tp + i
                for ct in range(NCT):
                    nc.tensor.matmul(
                        wide[:, i * 512:(i + 1) * 512],
                        xT[:, ct, tt * 128:(tt + 1) * 128],
                        wv_sb[:, ct, :],
                        start=(ct == 0), stop=(ct == NCT - 1))
            nc.vector.tensor_copy(
                v_aug[:, :, 2 * tp:2 * tp + 2, 0:64],
                wide[:].rearrange("p (i h d) -> p h i d", i=2, h=H_PER_CORE))

        def gen_q(qsb):
            """qT (padded layout) for q in [qsb*512, (qsb+1)*512)."""
            ring = qsb % 2
            for w in range(2):
                wide = ps.tile([128, 1024], F32, tag="wide", bufs=3)
                for i in range(2):
                    mt = 2 * w + i
                    for ct in range(NCT):
                        nc.tensor.matmul(
                            wide[:, i * 512:(i + 1) * 512],
                            wq_sb[:, ct, mt * 128:(mt + 1) * 128],
                            xT[:, ct, qsb * 512:(qsb + 1) * 512],
                            start=(ct == 0), stop=(ct == NCT - 1))
                # heads 4w,4w+2 live in partitions 0:64; 4w+1,4w+3 in 64:128
                src = wide[:].rearrange("p (i t) -> p i t", i=2)
                nc.vector.tensor_copy(
                    qT_ring[0:64, ring, 4 * w:4 * w + 3:2, :], src[0:64])
                nc.vector.tensor_copy(
                    qT_ring[64:128, ring, 4 * w + 1:4 * w + 4:2, :], src[64:128])

        def attention(qsb, fillers=()):
            fillers = list(fillers)
            ring = qsb % 2
            nkb = 4 * (qsb + 1)
            pending_pv = [None]
            pending_div = [None]

            def flush_pv():
                if pending_pv[0] is not None:
                    pending_pv[0]()
                    pending_pv[0] = None

            def flush_div():
                if pending_div[0] is not None:
                    pending_div[0]()
                    pending_div[0] = None

            for h in range(H_PER_CORE):
                mt_h = h // 2
                yTp = ps.tile([65, 512], F32, tag="yTp", bufs=2)
                prefix = [(kb, 0) for kb in range(4 * qsb)]
                diag = [(kb, kb * 128 - qsb * 512)
                        for kb in range(4 * qsb, nkb)]
                groups = [prefix[i:i + 2] for i in range(0, len(prefix), 2)]
                groups += [diag[0:2], diag[2:4]]
                n_pv = [0]
                for g in groups:
                    diag_group = g[0][0] >= 4 * qsb
                    wide = ps.tile([128, 1024], F32, tag="wide", bufs=3)
                    pTw = pT_pool.tile([128, 1024], F16, tag="pTw")
                    # pack blocks tightly: diag blocks are width-trimmed to
                    # their causal range and packed back-to-back so one exp
                    # ACTIVATE covers the pair with no padding columns.
                    offs, widths = [], []
                    off = 0
                    for kb, lo in g:
                        offs.append(off)
                        widths.append(512 - lo)
                        off += 512 - lo
                    for i, (kb, lo) in enumerate(g):
                        nc.tensor.matmul(
                            wide[:, offs[i]:offs[i] + widths[i]],
                            kT_sb[:, mt_h, kb * 128:(kb + 1) * 128],
                            qT_ring[:, ring, h, lo:512],
                            start=True, stop=True)
                    nc.scalar.activation(
                        pTw[:, 0:off], wide[:, 0:off],
                        AF.Exp, bias=0.0, scale=SCALE)
                    if diag_group:
                        for i in range(len(g)):
                            # zero the strictly-upper triangle (first 128
                            # cols of the trimmed block)
                            nc.vector.tensor_mul(
                                pTw[:, offs[i]:offs[i] + 128],
                                pTw[:, offs[i]:offs[i] + 128],
                                tri_mask[:])

                    def pv(g=g, offs=offs, widths=widths, pTw=pTw, yTp=yTp,
                           h=h, n_pv=n_pv):
                        for i, (kb, lo) in enumerate(g):
                            nc.tensor.matmul(
                                yTp[:, lo:512],
                                v_aug[:, h, kb, :],
                                pTw[:, offs[i]:offs[i] + widths[i]],
                                start=(n_pv[0] == 0),
                                stop=(n_pv[0] == nkb - 1))
                            n_pv[0] += 1

                    # one-group lag: emit the previous group's p@v (and any
                    # older division chain) after this group's scores
                    flush_pv()
                    flush_div()
                    pending_pv[0] = pv

                def div(h=h, mt_h=mt_h, yTp=yTp):
                    # softmax division straight off psum; yTp is freed by
                    # the final mul.
                    dn = rc_pool.tile([1, 512], F32, tag="dn")
                    nc.vector.tensor_copy(dn[:], yTp[64:65, :])
                    recip = rc_pool.tile([1, 512], F32, tag="recip")
                    nc.vector.reciprocal_approx_fast(recip[:], dn[:])
                    rbc = rb_pool.tile([64, 512], F32)
                    nc.gpsimd.partition_broadcast(rbc[:], recip[:])
                    nc.vector.tensor_mul(
                        yT_sb[64 * (h % 2):64 * (h % 2) + 64, mt_h,
                              qsb * 512:(qsb + 1) * 512],
                        yTp[0:64, :], rbc[:])

                pending_div[0] = div
                if fillers:
                    # filler PE work lands between this head's last scores
                    # and its pending p@v, covering the exp latency
                    fillers.pop(0)()
            flush_pv()
            flush_div()

        def proj(qsb):
            """out rows [qsb*512, (qsb+1)*512) = yT^T wp."""
            for j in range(4):
                tt = qsb * 4 + j
                wide = ps.tile([128, 1024], F32, tag="wide", bufs=3)
                for n2 in range(2):
                    for kt in range(4):
                        nc.tensor.matmul(
                            wide[:, n2 * 512:(n2 + 1) * 512],
                            yT_sb[:, kt, tt * 128:(tt + 1) * 128],
                            wp_sb[:, kt, n2 * 512:(n2 + 1) * 512],
                            start=(kt == 0), stop=(kt == 3))
                so = so_pool.tile([128, C], F32)
                if so_cnt[0] % 2 == 0:
                    nc.scalar.copy(so[:], wide[:])
                else:
                    nc.vector.tensor_copy(so[:], wide[:])
                so_cnt[0] += 1
                nc.sync.dma_start(
                    out_d[tt * 128:(tt + 1) * 128, :], so[:])

        # ---- emission schedule (software pipeline) ----
        gen_k(0, [0, 1, 2, 3])
        gen_v(0)
        gen_v(1)
        gen_q(0)
        attention(0, fillers=[
            lambda: gen_q(1),
            lambda: gen_k(1, [0]),
            lambda: gen_k(1, [1]),
            lambda: gen_k(1, [2]),
            lambda: gen_k(1, [3]),
            lambda: gen_v(2),
            lambda: gen_v(3),
        ])
        proj(0)
        attention(1, fillers=[
            lambda: gen_q(2),
            lambda: gen_v(4),
            lambda: gen_v(5),
            lambda: gen_v(6),
            lambda: gen_v(7),
        ])
        proj(1)
        attention(2, fillers=[lambda: gen_q(3)])
        proj(2)
        attention(3)
        proj(3)


_CACHE = {}


def _get_nc():
    if "nc" not in _CACHE:
        nc = bacc.Bacc("TRN2", target_bir_lowering=False, debug=False,
                       num_devices=N_CORES)
        build(nc)
        nc.compile()
        _CACHE["nc"] = nc
    return _CACHE["nc"]


def make_in_maps(x, w_attn, w_proj):
    x = np.asarray(x, dtype=np.float32)
    w_attn = np.asarray(w_attn, dtype=np.float32)
    w_proj = np.asarray(w_proj, dtype=np.float32)
    BF = ml_dtypes.bfloat16
    in_maps = []
    for core in range(N_CORES):
        b, hg = divmod(core, 2)
        cs = slice(hg * GC, (hg + 1) * GC)
        in_maps.append({
            "xT": np.ascontiguousarray(x[b].T.astype(BF)),
            "wq": np.ascontiguousarray(w_attn[:, 0 * C:1 * C][:, cs].astype(BF)),
            "wk": np.ascontiguousarray(w_attn[:, 1 * C:2 * C][:, cs].astype(BF)),
            "wv": np.ascontiguousarray(w_attn[:, 2 * C:3 * C][:, cs].astype(BF)),
            "wp": np.ascontiguousarray(w_proj[cs, :].astype(BF)),
        })
    return in_maps


def kernel(x, w_attn, w_proj, _trace=False, _trace_kwargs=None):
    nc = _get_nc()
    in_maps = make_in_maps(x, w_attn, w_proj)
    res = None
    for attempt in range(3):
        try:
            res = run_bass_kernel_spmd(nc, in_maps,
                                       core_ids=list(range(N_CORES)),
                                       trace=_trace, **(_trace_kwargs or {}))
            break
        except Exception:
            # a previous process can leave the device wedged
            # (NRT_EXEC_UNIT_UNRECOVERABLE); a retry recovers it
            if attempt == 2:
                raise
    _CACHE["last_results"] = res
    B = np.asarray(x).shape[0]
    out = np.empty((B, T, C), dtype=np.float32)
    for b in range(B):
        out[b] = res.results[2 * b]["out"] + res.results[2 * b + 1]["out"]
    return out


# revision 10
# speedup vs baseline: 2.3053x; 1.0630x over previous
"""Causal self-attention (B=4, T=2048, C=1024, H=16) on 8 TRN2 NeuronCores.

Sharding: core = (batch b, head-group hg). Data parallel over B (4), tensor
parallel over heads (2 groups of 8). Each core computes a partial output
projection for its 8 heads; the host sums the two partials per batch
(row-parallel linear unshard).

v3 design (vs the fp32r baseline):
  - x is transposed and cast to bf16 on the HOST (xT [C, T]); all matmul
    operands are bf16/f16 (fp32 psum accumulate).  This removes the 128 PE
    transposes, halves input DMA, and enables FWL fast weight loads.
  - Software-pipelined emission: k/v/q generation, attention per 512-wide q
    superblock (qsb), and the output projection are interleaved so the
    scalar-engine exp stream overlaps the qkv/proj matmuls.  Leftover
    generation work is emitted as "fillers" between attention heads so the
    PE never starves while exp catches up.
  - Cross-group pipelining inside attention: the p@v matmuls of group g are
    emitted after the scores matmuls of group g+1, so the exp->mask latency
    hides behind real PE work instead of stalling the in-order PE queue.
  - Diagonal score blocks are packed tightly in the psum wide tile so one
    exp ACTIVATE covers a whole pair with no padding columns.
  - Engines: scalar = exp only; vector = staging casts, masks, softmax
    division; gpsimd = partition broadcasts + memsets only (its FIFO is
    shallow and slow to resume, so it must never sit between PE-critical
    producers and consumers).

Per-core pipeline (per head h of 8, head dim 64, all psum fp32):
  kT [512ch, T] and qT (zero-padded per head to K=128) from wk/wq^T xT,
  v_aug [T, 8h, 65] (ones column -> softmax denominator from the PE),
  scoresT [k, q] = kT^T qT per 128-k-block, p = exp(scoresT/32) (|s|<=~2.1,
  no max subtraction), causal: skip blocks above the diagonal, width-trim +
  triangular-mask the 4 diagonal blocks, yT_aug [65, 512q] += v_aug^T p,
  yT = yT_aug[0:64] * (1/denominator), out [T, 1024] = yT^T wp.
"""
import numpy as np
from contextlib import ExitStack

import concourse.bass as bass
import concourse.mybir as mybir
import concourse.tile as tile
from concourse import bacc
from concourse.bass_utils import run_bass_kernel_spmd

import ml_dtypes

F32 = mybir.dt.float32
BF16 = mybir.dt.bfloat16
F16 = mybir.dt.float16
AF = mybir.ActivationFunctionType

T = 2048
C = 1024
H_PER_CORE = 8          # heads per core
D = 64                  # head dim
GC = H_PER_CORE * D     # 512 channels per head-group
SCALE = 1.0 / 32.0      # C ** -0.5
N_CORES = 8
NCT = C // 128          # 8 c-tiles
NT = T // 128           # 16 t-tiles
NQSB = T // 512         # 4 q superblocks


def build(nc):
    xT_d = nc.dram_tensor("xT", [C, T], BF16, kind="ExternalInput").ap()
    wq_d = nc.dram_tensor("wq", [C, GC], BF16, kind="ExternalInput").ap()
    wk_d = nc.dram_tensor("wk", [C, GC], BF16, kind="ExternalInput").ap()
    wv_d = nc.dram_tensor("wv", [C, GC], BF16, kind="ExternalInput").ap()
    wp_d = nc.dram_tensor("wp", [GC, C], BF16, kind="ExternalInput").ap()
    out_d = nc.dram_tensor("out", [T, C], F32, kind="ExternalOutput").ap()

    with tile.TileContext(nc) as tc, ExitStack() as ctx:
        const = ctx.enter_context(tc.tile_pool(name="const", bufs=1))
        persist = ctx.enter_context(tc.tile_pool(name="persist", bufs=1))

        # tri_mask[k, j] = 1.0 if k <= j else 0.0
        tri_mask = const.tile([128, 128], F16)
        nc.gpsimd.memset(tri_mask[:], 1.0)
        nc.gpsimd.affine_select(
            out=tri_mask[:], in_=tri_mask[:],
            compare_op=mybir.AluOpType.is_ge, fill=0.0, base=0,
            pattern=[[1, 128]], channel_multiplier=-1,
        )

        # persistent activations / weights
        xT = persist.tile([128, NCT, T], BF16)             # [c-tile, t]
        kT_sb = persist.tile([128, 4, T], BF16)            # [m-tile, t]
        v_aug = persist.tile([128, H_PER_CORE, NT, 65], F16)
        # q ring: slot = qsb % 2; head h occupies partition half (h%2)*64,
        # the other half stays zero (K=128 scores matmul, pad rows kill the
        # other head's kT contribution).
        qT_ring = persist.tile([128, 2, H_PER_CORE, 512], BF16)
        yT_sb = persist.tile([128, 4, T], BF16)
        wq_sb = persist.tile([128, NCT, GC], BF16)
        wk_sb = persist.tile([128, NCT, GC], BF16)
        wv_sb = persist.tile([128, NCT, GC], BF16)
        wp_sb = persist.tile([128, 4, C], BF16)

        nc.gpsimd.memset(qT_ring[:], 0.0)
        nc.gpsimd.memset(v_aug[:, :, :, 64], 1.0)

        # input DMAs; wk/xT interleaved so k-gen can start ASAP.  xT tiles
        # come in t-halves so the first k/v matmuls start ~4us earlier.
        for ct in range(NCT):
            nc.sync.dma_start(wk_sb[:, ct, :], wk_d[ct * 128:(ct + 1) * 128, :])
            nc.sync.dma_start(xT[:, ct, 0:1024],
                              xT_d[ct * 128:(ct + 1) * 128, 0:1024])
        for ct in range(NCT):
            nc.sync.dma_start(xT[:, ct, 1024:2048],
                              xT_d[ct * 128:(ct + 1) * 128, 1024:2048])
        for ct in range(NCT):
            nc.sync.dma_start(wv_sb[:, ct, :], wv_d[ct * 128:(ct + 1) * 128, :])
        for ct in range(NCT):
            nc.sync.dma_start(wq_sb[:, ct, :], wq_d[ct * 128:(ct + 1) * 128, :])
        for kt in range(4):
            nc.sync.dma_start(wp_sb[:, kt, :], wp_d[kt * 128:(kt + 1) * 128, :])

        pT_pool = ctx.enter_context(tc.tile_pool(name="pT", bufs=6))
        rc_pool = ctx.enter_context(tc.tile_pool(name="rc", bufs=3))
        rb_pool = ctx.enter_context(tc.tile_pool(name="rb", bufs=3))
        so_pool = ctx.enter_context(tc.tile_pool(name="so", bufs=3))
        ps = ctx.enter_context(tc.tile_pool(name="ps", bufs=1, space="PSUM"))
        # psum budget: wide 3*[128,1024] (6 banks) + yTp 2*[65,512] (2 banks)


        def gen_k(half, mts):
            """kT for t in [half*1024, (half+1)*1024) for the given m-tiles."""
            for mt in mts:
                wide = ps.tile([128, 1024], F32, tag="wide", bufs=3)
                for c2 in range(2):
                    chunk = half * 2 + c2
                    for ct in range(NCT):
                        nc.tensor.matmul(
                            wide[:, c2 * 512:(c2 + 1) * 512],
                            wk_sb[:, ct, mt * 128:(mt + 1) * 128],
                            xT[:, ct, chunk * 512:(chunk + 1) * 512],
                            start=(ct == 0), stop=(ct == NCT - 1))
                nc.vector.tensor_copy(
                    kT_sb[:, mt, half * 1024:(half + 1) * 1024], wide[:])

        def gen_v(tp):
            """v for t-tiles 2*tp, 2*tp+1 into v_aug."""
            wide = ps.tile([128, 1024], F32, tag="wide", bufs=3)
            for i in range(2):
                tt = 2 * tp + i
                for ct in range(NCT):
                    nc.tensor.matmul(
                        wide[:, i * 512:(i + 1) * 512],
                        xT[:, ct, tt * 128:(tt + 1) * 128],
                        wv_sb[:, ct, :],
                        start=(ct == 0), stop=(ct == NCT - 1))
            nc.vector.tensor_copy(
                v_aug[:, :, 2 * tp:2 * tp + 2, 0:64],
                wide[:].rearrange("p (i h d) -> p h i d", i=2, h=H_PER_CORE))

        def gen_q(qsb):
            """qT (padded layout) for q in [qsb*512, (qsb+1)*512)."""
            ring = qsb % 2
            for w in range(2):
                wide = ps.tile([128, 1024], F32, tag="wide", bufs=3)
                for i in range(2):
                    mt = 2 * w + i
                    for ct in range(NCT):
                        nc.tensor.matmul(
                            wide[:, i * 512:(i + 1) * 512],
                            wq_sb[:, ct, mt * 128:(mt + 1) * 128],
                            xT[:, ct, qsb * 512:(qsb + 1) * 512],
                            start=(ct == 0), stop=(ct == NCT - 1))
                # heads 4w,4w+2 live in partitions 0:64; 4w+1,4w+3 in 64:128
                src = wide[:].rearrange("p (i t) -> p i t", i=2)
                nc.vector.tensor_copy(
                    qT_ring[0:64, ring, 4 * w:4 * w + 3:2, :], src[0:64])
                nc.vector.tensor_copy(
                    qT_ring[64:128, ring, 4 * w + 1:4 * w + 4:2, :], src[64:128])

        def attention(qsb, fillers=()):
            fillers = list(fillers)
            ring = qsb % 2
            nkb = 4 * (qsb + 1)
            pending_pv = [None]
            pending_div = [None]

            def flush_pv():
                if pending_pv[0] is not None:
                    pending_pv[0]()
                    pending_pv[0] = None

            def flush_div():
                if pending_div[0] is not None:
                    pending_div[0]()
                    pending_div[0] = None

            for h in range(H_PER_CORE):
                mt_h = h // 2
                yTp = ps.tile([65, 512], F32, tag="yTp", bufs=2)
                prefix = [(kb, 0) for kb in range(4 * qsb)]
                diag = [(kb, kb * 128 - qsb * 512)
                        for kb in range(4 * qsb, nkb)]
                groups = [prefix[i:i + 2] for i in range(0, len(prefix), 2)]
                groups += [diag[0:2], diag[2:4]]
                n_pv = [0]
                for g in groups:
                    diag_group = g[0][0] >= 4 * qsb
                    wide = ps.tile([128, 1024], F32, tag="wide", bufs=3)
                    pTw = pT_pool.tile([128, 1024], F16, tag="pTw")
                    # pack blocks tightly: diag blocks are width-trimmed to
                    # their causal range and packed back-to-back so one exp
                    # ACTIVATE covers the pair with no padding columns.
                    offs, widths = [], []
                    off = 0
                    for kb, lo in g:
                        offs.append(off)
                        widths.append(512 - lo)
                        off += 512 - lo
                    for i, (kb, lo) in enumerate(g):
                        nc.tensor.matmul(
                            wide[:, offs[i]:offs[i] + widths[i]],
                            kT_sb[:, mt_h, kb * 128:(kb + 1) * 128],
                            qT_ring[:, ring, h, lo:512],
                            start=True, stop=True)
                    nc.scalar.activation(
                        pTw[:, 0:off], wide[:, 0:off],
                        AF.Exp, bias=0.0, scale=SCALE)
                    if diag_group:
                        for i in range(len(g)):
                            # zero the strictly-upper triangle (first 128
                            # cols of the trimmed block)
                            nc.vector.tensor_mul(
                                pTw[:, offs[i]:offs[i] + 128],
                                pTw[:, offs[i]:offs[i] + 128],
                                tri_mask[:])

                    def pv(g=g, offs=offs, widths=widths, pTw=pTw, yTp=yTp,
                           h=h, n_pv=n_pv):
                        for i, (kb, lo) in enumerate(g):
                            nc.tensor.matmul(
                                yTp[:, lo:512],
                                v_aug[:, h, kb, :],
                                pTw[:, offs[i]:offs[i] + widths[i]],
                                start=(n_pv[0] == 0),
                                stop=(n_pv[0] == nkb - 1))
                            n_pv[0] += 1

                    # one-group lag: emit the previous group's p@v (and any
                    # older division chain) after this group's scores
                    flush_pv()
                    flush_div()
                    pending_pv[0] = pv

                def div(h=h, mt_h=mt_h, yTp=yTp):
                    # softmax division straight off psum; yTp is freed by
                    # the final mul.
                    dn = rc_pool.tile([1, 512], F32, tag="dn")
                    nc.vector.tensor_copy(dn[:], yTp[64:65, :])
                    recip = rc_pool.tile([1, 512], F32, tag="recip")
                    nc.vector.reciprocal_approx_fast(recip[:], dn[:])
                    rbc = rb_pool.tile([64, 512], F32)
                    nc.gpsimd.partition_broadcast(rbc[:], recip[:])
                    nc.vector.tensor_mul(
                        yT_sb[64 * (h % 2):64 * (h % 2) + 64, mt_h,
                              qsb * 512:(qsb + 1) * 512],
                        yTp[0:64, :], rbc[:])

                pending_div[0] = div
                if fillers:
                    # filler PE work lands between this head's last scores
                    # and its pending p@v, covering the exp latency
                    fillers.pop(0)()
            flush_pv()
            flush_div()
            for f in fillers:
                f()

        def proj(qsb, j):
            """out rows for t-tile qsb*4+j = yT^T wp."""
            tt = qsb * 4 + j
            wide = ps.tile([128, 1024], F32, tag="wide", bufs=3)
            for n2 in range(2):
                for kt in range(4):
                    nc.tensor.matmul(
                        wide[:, n2 * 512:(n2 + 1) * 512],
                        yT_sb[:, kt, tt * 128:(tt + 1) * 128],
                        wp_sb[:, kt, n2 * 512:(n2 + 1) * 512],
                        start=(kt == 0), stop=(kt == 3))
            so = so_pool.tile([128, C], F32)
            nc.vector.tensor_copy(so[:], wide[:])
            nc.sync.dma_start(out_d[tt * 128:(tt + 1) * 128, :], so[:])

        # ---- emission schedule (software pipeline) ----
        # proj(qsb) runs as fillers inside attention(qsb+1) so it never
        # stalls on the last head's softmax-division chain.
        gen_k(0, [0, 1, 2, 3])
        gen_v(0)
        gen_v(1)
        gen_q(0)
        attention(0, fillers=[
            lambda: gen_q(1),
            lambda: gen_k(1, [0]),
            lambda: gen_k(1, [1]),
            lambda: gen_k(1, [2]),
            lambda: gen_k(1, [3]),
            lambda: gen_v(2),
            lambda: gen_v(3),
        ])
        attention(1, fillers=[
            lambda: gen_q(2),
            lambda: proj(0, 0),
            lambda: proj(0, 1),
            lambda: proj(0, 2),
            lambda: proj(0, 3),
            lambda: gen_v(4),
            lambda: gen_v(5),
            lambda: gen_v(6),
            lambda: gen_v(7),
        ])
        attention(2, fillers=[
            lambda: gen_q(3),
            lambda: proj(1, 0),
            lambda: proj(1, 1),
            lambda: proj(1, 2),
            lambda: proj(1, 3),
        ])
        attention(3, fillers=[
            lambda: proj(2, 0),
            lambda: proj(2, 1),
            lambda: proj(2, 2),
            lambda: proj(2, 3),
        ])
        for j in range(4):
            proj(3, j)


_CACHE = {}


def _get_nc():
    if "nc" not in _CACHE:
        nc = bacc.Bacc("TRN2", target_bir_lowering=False, debug=False,
                       num_devices=N_CORES)
        build(nc)
        nc.compile()
        _CACHE["nc"] = nc
    return _CACHE["nc"]


def make_in_maps(x, w_attn, w_proj):
    x = np.asarray(x, dtype=np.float32)
    w_attn = np.asarray(w_attn, dtype=np.float32)
    w_proj = np.asarray(w_proj, dtype=np.float32)
    BF = ml_dtypes.bfloat16
    in_maps = []
    for core in range(N_CORES):
        b, hg = divmod(core, 2)
        cs = slice(hg * GC, (hg + 1) * GC)
        in_maps.append({
            "xT": np.ascontiguousarray(x[b].T.astype(BF)),
            "wq": np.ascontiguousarray(w_attn[:, 0 * C:1 * C][:, cs].astype(BF)),
            "wk": np.ascontiguousarray(w_attn[:, 1 * C:2 * C][:, cs].astype(BF)),
            "wv": np.ascontiguousarray(w_attn[:, 2 * C:3 * C][:, cs].astype(BF)),
            "wp": np.ascontiguousarray(w_proj[cs, :].astype(BF)),
        })
    return in_maps


def kernel(x, w_attn, w_proj, _trace=False, _trace_kwargs=None):
    nc = _get_nc()
    in_maps = make_in_maps(x, w_attn, w_proj)
    res = None
    for attempt in range(3):
        try:
            res = run_bass_kernel_spmd(nc, in_maps,
                                       core_ids=list(range(N_CORES)),
                                       trace=_trace, **(_trace_kwargs or {}))
            break
        except Exception:
            # a previous process can leave the device wedged
            # (NRT_EXEC_UNIT_UNRECOVERABLE); a retry recovers it
            if attempt == 2:
                raise
    _CACHE["last_results"] = res
    B = np.asarray(x).shape[0]
    out = np.empty((B, T, C), dtype=np.float32)
    for b in range(B):
        out[b] = res.results[2 * b]["out"] + res.results[2 * b + 1]["out"]
    return out
